# revision 13
# baseline (speedup 1.0000x reference)
"""Trainium2 Bass kernel for nn_DescriptorNetwork (gnn_message_passing).

Sharding: pure data parallelism over crystals. Each of 8 cores processes 1250
crystals (padded to 1280). Edges are all intra-crystal 8x8 pairs and nodes are
contiguous per crystal, so every gather/segment op is structural: fixed 0/1
selection matmuls and access patterns; no dynamic indexing on device.

Math notes (validated against the jax reference in a numpy golden model):
- gate second layer folded into the activation: W1g pre-scaled by w2 per hidden
  unit (sign-permuted), Lrelu evaluated with alpha=0.01 for w2>0 units and
  alpha=100 for w2<0 units (identity  w2*lrelu_a(y) = a*lrelu_{1/a}(w2*y) for
  w2<0), then ACT's accum_out reduces over the hidden dim for free.
- softmax without max subtraction (logits are O(1); exp safe in fp32); gate
  bias b2g cancels in the softmax.  Normalization folded into the G matrix, so
  sum(g)=1 exactly and the msg bias b2m is a plain add after W2.
- denominator via a constant same-segment mask matmul over a (128,16) batch of
  gate columns.
"""

import numpy as np

N_CRY = 10000
N_EL = 8
N = N_CRY * N_EL
FEA = 64
HID = 256
NCORE = 8
C_CORE = N_CRY // NCORE     # 1250
C_PAD = 1280
NN = C_PAD * N_EL           # 10240 nodes/core
N_GROUP = C_PAD // 8        # 160
N_ETILE = N_GROUP * 4       # 640 edge tiles
N_NTILE = NN // 128         # 80 node tiles
BATCH_ET = 16
N_BATCH = N_ETILE // BATCH_ET   # 40
ALPHA = 0.01

_CACHE = {}


# ---------------- host-side preprocessing ----------------

def _prep_head(head):
    W1g, b1g = [np.asarray(x, np.float32) for x in head["gate"][0]]
    w2g, b2g = [np.asarray(x, np.float32) for x in head["gate"][1]]
    W1m, b1m = [np.asarray(x, np.float32) for x in head["msg"][0]]
    W2m, b2m = [np.asarray(x, np.float32) for x in head["msg"][1]]
    pow_ = float(np.asarray(head["pow"]).reshape(-1)[0])
    w2 = w2g[:, 0]
    order = np.argsort(w2 <= 0, kind="stable")
    n_pos = int((w2 > 0).sum())
    w2a = np.abs(w2)
    W1gs = W1g[:, order] * w2a[order][None, :]
    b1gs = b1g[order] * w2a[order]
    return dict(W1g=W1gs, b1g=b1gs, n_pos=n_pos,
                W1m=W1m, b1m=b1m, W2m=W2m, b2m=b2m, pow=pow_)


def _sel_matrix():
    sel = np.zeros((128, 512), np.float32)
    for t in range(4):
        for p in range(128):
            k_self = 16 * t + 8 * (p // 64) + (p % 64) // 8
            k_nbr = 64 + 16 * t + 8 * (p // 64) + (p % 8)
            sel[k_self, 128 * t + p] = 1.0
            sel[k_nbr, 128 * t + p] = 1.0
    return sel


def _segmask():
    m = np.zeros((128, 128), np.float32)
    for e in range(128):
        s = e // 8
        m[s * 8:(s + 1) * 8, e] = 1.0
    return m


def _gmasks():
    """(128, 64): [:,0:32] even-tile mask (local cols 0..15), [:,32:64] odd
    (local cols 16..31)."""
    g = np.zeros((128, 64), np.float32)
    for e in range(128):
        g[e, e // 8] = 1.0
        g[e, 32 + 16 + e // 8] = 1.0
    return g


def _edge_wp(w_core, pow_):
    NE = C_PAD * 64
    wp = np.ones(NN, np.float32)
    wp[:C_CORE * 8] = np.asarray(w_core, np.float32).reshape(-1) ** np.float32(pow_)
    e = np.broadcast_to(wp.reshape(C_PAD, 1, 8), (C_PAD, 8, 8)).reshape(NE)
    return np.ascontiguousarray(e.reshape(N_ETILE, 128).T)


def _node_wp(w_core, pow_):
    wp = np.ones(NN, np.float32)
    wp[:C_CORE * 8] = np.asarray(w_core, np.float32).reshape(-1) ** np.float32(pow_)
    return np.ascontiguousarray(wp.reshape(N_NTILE, 128).T)


def _prep_host(inputs):
    p = inputs["params"]
    We, be = [np.asarray(x, np.float32) for x in p["elem_embed"]]
    Ws, bs = [np.asarray(x, np.float32) for x in p["sym_embed"]]
    heads = [_prep_head(layer["heads"][0]) for layer in p["graphs"]]
    cry = _prep_head(p["cry"][0])

    elem_fea = np.asarray(inputs["elem_fea"], np.float32)
    sym_fea = np.asarray(inputs["sym_fea"], np.float32)
    w = np.asarray(inputs["elem_weights"], np.float32)
    symw = np.concatenate([sym_fea, w], axis=1)

    consts = {
        "sel": _sel_matrix(),
        "segmask": _segmask(),
        "gmask": _gmasks(),
        "ident": np.eye(128, dtype=np.float32),
        "emb_bias": np.concatenate([be, bs]).reshape(64, 1).astype(np.float32),
        "We": We, "Ws": Ws,
    }
    for li, h in enumerate(heads):
        consts[f"wns_{li}"] = np.concatenate([h["W1g"][:64], h["W1m"][:64]], 1)
        consts[f"wnn_{li}"] = np.concatenate([
            np.concatenate([h["W1g"][64:], h["W1m"][64:]], 1),
            np.concatenate([h["b1g"], h["b1m"]]).reshape(1, 512)], 0)
        consts[f"w2p_{li}"] = np.concatenate([h["W2m"][:128], h["W2m"][128:]], 1)
        consts[f"b2f_{li}"] = np.broadcast_to(h["b2m"], (128, 64)).copy()
    consts["wc"] = np.concatenate([
        np.concatenate([cry["W1g"][:64], cry["W1m"][:64]], 1),
        np.concatenate([cry["b1g"], cry["b1m"]]).reshape(1, 512)], 0)
    consts["w2pc"] = np.concatenate([cry["W2m"][:128], cry["W2m"][128:]], 1)
    consts["b2fc"] = np.broadcast_to(cry["b2m"], (128, 64)).copy()

    n_pos = [h["n_pos"] for h in heads] + [cry["n_pos"]]

    per_core = []
    for k in range(NCORE):
        ns = slice(10000 * k, 10000 * (k + 1))
        elemT = np.zeros((200, NN), np.float32)
        elemT[:, :10000] = elem_fea[ns].T
        symT = np.zeros((445, NN), np.float32)
        symT[:, :10000] = symw[ns].T
        d = {"elemT": elemT, "symT": symT}
        for li, h in enumerate(heads):
            d[f"wp_{li}"] = _edge_wp(w[ns], h["pow"])
        d["wpc"] = _node_wp(w[ns], cry["pow"])
        per_core.append(d)
    return consts, per_core, n_pos


# ---------------- device kernel builder ----------------

def _build(n_pos_list, use_bf16):
    import concourse.bass as bass
    import concourse.tile as tile
    from concourse import bacc, mybir

    f32 = mybir.dt.float32
    bf16 = mybir.dt.bfloat16
    mdt = bf16 if use_bf16 else f32
    AF = mybir.ActivationFunctionType
    ALU = mybir.AluOpType
    ts = bass.ts

    nc = bacc.Bacc("TRN2", target_bir_lowering=False, debug=False,
                   enable_asserts=False, num_devices=NCORE)

    def dram_in(name, shape):
        return nc.dram_tensor(name, list(shape), f32, kind="ExternalInput")

    elemT_d = dram_in("elemT", (200, NN))
    symT_d = dram_in("symT", (445, NN))
    sel_d = dram_in("sel", (128, 512))
    segmask_d = dram_in("segmask", (128, 128))
    gmask_d = dram_in("gmask", (128, 64))
    ident_d = dram_in("ident", (128, 128))
    emb_bias_d = dram_in("emb_bias", (64, 1))
    We_d = dram_in("We", (200, 32))
    Ws_d = dram_in("Ws", (445, 32))
    layer_d = []
    for li in range(3):
        layer_d.append({k: dram_in(f"{k}_{li}", s) for k, s in
                        [("wns", (64, 512)), ("wnn", (65, 512)),
                         ("w2p", (128, 128)), ("b2f", (128, 64)),
                         ("wp", (128, N_ETILE))]})
    wc_d = dram_in("wc", (65, 512))
    w2pc_d = dram_in("w2pc", (128, 128))
    b2fc_d = dram_in("b2fc", (128, 64))
    wpc_d = dram_in("wpc", (128, N_NTILE))
    out_d = nc.dram_tensor("out", [C_PAD, 64], f32, kind="ExternalOutput")

    from contextlib import ExitStack
    ctx = ExitStack()
    with tile.TileContext(nc) as tc, ctx:
        cpool = ctx.enter_context(tc.tile_pool(name="consts", bufs=1))

        _cnt = [0]

        def load_const(dram, shape, dtype=f32, rows=None):
            _cnt[0] += 1
            nm = f"c{_cnt[0]}_{dram.name}"
            t = cpool.tile(list(shape), dtype, name=nm, tag=nm)
            src = dram[rows[0]:rows[1], :] if rows else dram[:]
            if dtype == f32:
                nc.sync.dma_start(t[:], src)
            else:
                tf = cpool.tile(list(shape), f32, name=nm + "f", tag="cstage")
                nc.sync.dma_start(tf[:], src)
                nc.vector.tensor_copy(t[:], tf[:])
            return t

        sel = load_const(sel_d, (128, 512), mdt)
        segmask = load_const(segmask_d, (128, 128))
        gmask = load_const(gmask_d, (128, 64), mdt)
        ident = load_const(ident_d, (128, 128))
        ident_m = load_const(ident_d, (128, 128), mdt) if use_bf16 else ident
        emb_bias = load_const(emb_bias_d, (64, 1))
        We1 = load_const(We_d, (128, 32), rows=(0, 128))
        We2 = load_const(We_d, (72, 32), rows=(128, 200))
        WsC = []
        for _si, (lo, hi) in enumerate([(0, 128), (128, 256), (256, 384), (384, 445)]):
            WsC.append(load_const(Ws_d, (hi - lo, 32), rows=(lo, hi)))
        LW = []
        for li in range(3):
            LW.append({
                "wns": load_const(layer_d[li]["wns"], (64, 512), mdt),
                "wnn": load_const(layer_d[li]["wnn"], (65, 512), mdt),
                "w2p": load_const(layer_d[li]["w2p"], (128, 128), mdt),
                "b2f": load_const(layer_d[li]["b2f"], (128, 64)),
                "wp": load_const(layer_d[li]["wp"], (128, N_ETILE)),
            })
        wc = load_const(wc_d, (65, 512), mdt)
        w2pc = load_const(w2pc_d, (128, 128), mdt)
        b2fc = load_const(b2fc_d, (128, 64))
        wpc = load_const(wpc_d, (128, N_NTILE))

        feaT_a = cpool.tile([65, NN], mdt, tag="feaT_a")
        fea_a = cpool.tile([128, N_NTILE * 64], f32, tag="fea_a")
        nc.vector.memset(feaT_a[64:65, :], 1.0)

        iopool = ctx.enter_context(tc.tile_pool(name="io", bufs=2))
        psA = ctx.enter_context(tc.tile_pool(name="psA", bufs=3, space="PSUM"))
        psB = ctx.enter_context(tc.tile_pool(name="psB", bufs=3, space="PSUM"))
        psC = ctx.enter_context(tc.tile_pool(name="psC", bufs=2, space="PSUM"))
        work = ctx.enter_context(tc.tile_pool(name="work", bufs=3))
        sigp = ctx.enter_context(tc.tile_pool(name="sig", bufs=BATCH_ET + 2))

        # ---------------- embed ----------------
        NCHUNK = NN // 512
        for ch in range(NCHUNK):
            cols = ts(ch, 512)
            et = iopool.tile([128, 512], f32, tag="elemT")
            et2 = iopool.tile([72, 512], f32, tag="elemT2")
            nc.sync.dma_start(et[:], elemT_d[0:128, cols])
            nc.sync.dma_start(et2[:], elemT_d[128:200, cols])
            sts = []
            for si, (lo, hi) in enumerate([(0, 128), (128, 256), (256, 384), (384, 445)]):
                st = iopool.tile([hi - lo, 512], f32, tag=f"symT{si}")
                nc.sync.dma_start(st[:], symT_d[lo:hi, cols])
                sts.append(st)
            ep = psA.tile([128, 512], f32, tag="big")
            nc.tensor.matmul(ep[0:32, :], We1[:], et[:], start=True, stop=False)
            nc.tensor.matmul(ep[0:32, :], We2[:], et2[:], start=False, stop=True)
            for si in range(4):
                nc.tensor.matmul(ep[32:64, :], WsC[si][:], sts[si][:],
                                 start=(si == 0), stop=(si == 3))
            nc.scalar.activation(feaT_a[0:64, cols], ep[0:64, :], AF.Identity,
                                 bias=emb_bias[:])
        for nt in range(N_NTILE):
            tp = psC.tile([128, 128], mdt, tag="tr")
            nc.tensor.transpose(tp[0:128, 0:64], feaT_a[0:64, ts(nt, 128)],
                                ident_m[0:64, 0:64])
            nc.vector.tensor_copy(fea_a[:, ts(nt, 64)], tp[0:128, 0:64])

        # ---------------- message passing layers ----------------
        feaT_cur, fea_cur, feaT_nxt, fea_nxt = feaT_a, fea_a, feaT_a, fea_a
        for li in range(3):
            L = LW[li]
            npos = n_pos_list[li]
            for b in range(N_BATCH):
                gp = work.tile([128, BATCH_ET], f32, tag="gpos")
                gn = work.tile([128, BATCH_ET], f32, tag="gneg")
                sig_tiles = []
                for gg in range(4):
                    g = 4 * b + gg
                    stp = psA.tile([128, 512], f32, tag="big")
                    ncols = ts(g, 64)
                    nc.tensor.matmul(stp[0:64, :], feaT_cur[0:64, ncols],
                                     L["wns"][:], start=True, stop=True)
                    nc.tensor.matmul(stp[64:128, :], feaT_cur[0:65, ncols],
                                     L["wnn"][:], start=True, stop=True)
                    stk = work.tile([128, 512], mdt, tag="stk_sb")
                    nc.scalar.copy(stk[:, 0:256], stp[:, 0:256])
                    nc.vector.tensor_copy(stk[:, 256:512], stp[:, 256:512])
                    for t in range(4):
                        bt = 4 * gg + t
                        ep = psA.tile([128, 512], f32, tag="big")
                        nc.tensor.matmul(ep[:], sel[:, ts(t, 128)], stk[:],
                                         start=True, stop=True)
                        scr = work.tile([128, 256], mdt, tag="scr")
                        if npos > 0:
                            nc.scalar.activation(
                                scr[:, 0:npos], ep[:, 0:npos], AF.Lrelu,
                                alpha=ALPHA, accum_out=gp[:, bt:bt + 1])
                        else:
                            nc.vector.memset(gp[:, bt:bt + 1], 0.0)
                        if npos < 256:
                            nc.scalar.activation(
                                scr[:, npos:256], ep[:, npos:256], AF.Lrelu,
                                alpha=ALPHA, accum_out=gn[:, bt:bt + 1])
                        else:
                            nc.vector.memset(gn[:, bt:bt + 1], 0.0)
                        sig = sigp.tile([128, 256], mdt, tag="sig")
                        nc.scalar.activation(sig[:], ep[:, 256:512], AF.Lrelu,
                                             alpha=ALPHA)
                        sig_tiles.append(sig)
                gam = work.tile([128, BATCH_ET], f32, tag="gam")
                nc.vector.tensor_sub(gam[:], gp[:], gn[:])
                gex = work.tile([128, BATCH_ET], f32, tag="gex")
                nc.scalar.activation(gex[:], gam[:], AF.Exp)
                gt = work.tile([128, BATCH_ET], f32, tag="gt")
                nc.vector.tensor_mul(gt[:], gex[:], L["wp"][:, ts(b, BATCH_ET)])
                dnp = psB.tile([128, 256], f32, tag="T")
                nc.tensor.matmul(dnp[:, 0:BATCH_ET], segmask[:], gt[:],
                                 start=True, stop=True)
                rec = work.tile([128, BATCH_ET], f32, tag="rec")
                nc.vector.reciprocal(rec[:], dnp[:, 0:BATCH_ET])
                gnm = work.tile([128, BATCH_ET], f32, tag="gnm")
                nc.vector.tensor_mul(gnm[:], gt[:], rec[:])
                Tps = [psB.tile([128, 256], f32, tag="T", name=f"Tp{_i}") for _i in range(2)]
                for t16 in range(BATCH_ET):
                    Gt = work.tile([128, 32], mdt, tag="G")
                    half = slice(0, 32) if t16 % 2 == 0 else slice(32, 64)
                    nc.vector.tensor_scalar_mul(Gt[:], gmask[:, half],
                                                gnm[:, t16:t16 + 1])
                    Tp = Tps[t16 // 8]
                    q = (t16 % 8) // 2
                    nc.tensor.matmul(Tp[32 * q:32 * (q + 1), :], Gt[:],
                                     sig_tiles[t16][:],
                                     start=(t16 % 2 == 0), stop=(t16 % 2 == 1),
                                     tile_position=(0, 32 * q))
                for ntb in range(2):
                    nt = 2 * b + ntb
                    Tsb = work.tile([128, 256], mdt, tag="Tsb")
                    nc.vector.tensor_copy(Tsb[:], Tps[ntb][:])
                    up = psC.tile([128, 128], f32, tag="tr")
                    for hh in range(2):
                        tp = psC.tile([128, 128], mdt, tag="tr")
                        nc.tensor.transpose(tp[:], Tsb[:, ts(hh, 128)], ident_m[:])
                        tsb = work.tile([128, 128], mdt, tag="TsbT")
                        nc.scalar.copy(tsb[:], tp[:])
                        nc.tensor.matmul(up[:, 0:64], tsb[:],
                                         L["w2p"][:, ts(hh, 64)],
                                         start=(hh == 0), stop=(hh == 1))
                    fcols = ts(nt, 64)
                    tmp = work.tile([128, 64], f32, tag="utmp")
                    nc.vector.tensor_add(tmp[:], up[:, 0:64], L["b2f"][:])
                    nc.vector.tensor_add(fea_nxt[:, fcols], tmp[:], fea_cur[:, fcols])
                    tp2 = psC.tile([128, 128], f32, tag="tr")
                    nc.tensor.transpose(tp2[0:64, :], fea_nxt[:, fcols], ident[:])
                    nc.scalar.copy(feaT_nxt[0:64, ts(nt, 128)], tp2[0:64, :])


        # ---------------- crystal pooling ----------------
        nposc = n_pos_list[3]
        NB_C = N_NTILE // BATCH_ET
        for b in range(NB_C):
            gp = work.tile([128, BATCH_ET], f32, tag="gpos")
            gn = work.tile([128, BATCH_ET], f32, tag="gneg")
            sig_tiles = []
            for bt in range(BATCH_ET):
                nt = BATCH_ET * b + bt
                ep = psA.tile([128, 512], f32, tag="big")
                nc.tensor.matmul(ep[:], feaT_cur[0:65, ts(nt, 128)], wc[:],
                                 start=True, stop=True)
                scr = work.tile([128, 256], mdt, tag="scr")
                if nposc > 0:
                    nc.scalar.activation(scr[:, 0:nposc], ep[:, 0:nposc], AF.Lrelu,
                                         alpha=ALPHA, accum_out=gp[:, bt:bt + 1])
                else:
                    nc.vector.memset(gp[:, bt:bt + 1], 0.0)
                if nposc < 256:
                    nc.scalar.activation(scr[:, nposc:256], ep[:, nposc:256],
                                         AF.Lrelu, alpha=ALPHA,
                                         accum_out=gn[:, bt:bt + 1])
                else:
                    nc.vector.memset(gn[:, bt:bt + 1], 0.0)
                sig = sigp.tile([128, 256], mdt, tag="sig")
                nc.scalar.activation(sig[:], ep[:, 256:512], AF.Lrelu,
                                     alpha=ALPHA)
                sig_tiles.append(sig)
            gam = work.tile([128, BATCH_ET], f32, tag="gam")
            nc.vector.tensor_sub(gam[:], gp[:], gn[:])
            gex = work.tile([128, BATCH_ET], f32, tag="gex")
            nc.scalar.activation(gex[:], gam[:], AF.Exp)
            gt = work.tile([128, BATCH_ET], f32, tag="gt")
            nc.vector.tensor_mul(gt[:], gex[:], wpc[:, ts(b, BATCH_ET)])
            dnp = psB.tile([128, 256], f32, tag="T")
            nc.tensor.matmul(dnp[:, 0:BATCH_ET], segmask[:], gt[:],
                             start=True, stop=True)
            rec = work.tile([128, BATCH_ET], f32, tag="rec")
            nc.vector.reciprocal(rec[:], dnp[:, 0:BATCH_ET])
            gnm = work.tile([128, BATCH_ET], f32, tag="gnm")
            nc.vector.tensor_mul(gnm[:], gt[:], rec[:])
            Tps = [psB.tile([128, 256], f32, tag="T", name=f"Tp{_i}") for _i in range(2)]
            for t16 in range(BATCH_ET):
                Gt = work.tile([128, 32], mdt, tag="G")
                half = slice(0, 32) if t16 % 2 == 0 else slice(32, 64)
                nc.vector.tensor_scalar_mul(Gt[:], gmask[:, half],
                                            gnm[:, t16:t16 + 1])
                Tp = Tps[t16 // 8]
                q = (t16 % 8) // 2
                nc.tensor.matmul(Tp[32 * q:32 * (q + 1), :], Gt[:],
                                 sig_tiles[t16][:],
                                 start=(t16 % 2 == 0), stop=(t16 % 2 == 1),
                                 tile_position=(0, 32 * q))
            for ntb in range(2):
                ct = 2 * b + ntb
                Tsb = work.tile([128, 256], mdt, tag="Tsb")
                nc.vector.tensor_copy(Tsb[:], Tps[ntb][:])
                up = psC.tile([128, 128], f32, tag="tr")
                for hh in range(2):
                    tp = psC.tile([128, 128], mdt, tag="tr")
                    nc.tensor.transpose(tp[:], Tsb[:, ts(hh, 128)], ident_m[:])
                    tsb = work.tile([128, 128], mdt, tag="TsbT")
                    nc.scalar.copy(tsb[:], tp[:])
                    nc.tensor.matmul(up[:, 0:64], tsb[:], w2pc[:, ts(hh, 64)],
                                     start=(hh == 0), stop=(hh == 1))
                ot = work.tile([128, 64], f32, tag="out")
                nc.vector.tensor_add(ot[:], up[:, 0:64], b2fc[:])
                nc.sync.dma_start(out_d[ts(ct, 128), :], ot[:])

    nc.compile()
    return nc


# ---------------- entry point ----------------

def kernel(**inputs) -> np.ndarray:
    from concourse.bass_utils import run_bass_kernel_spmd

    consts, per_core, n_pos = _prep_host(inputs)

    import os
    use_bf16 = bool(int(os.environ.get("KERNEL_BF16", "0")))
    key = ("k", tuple(n_pos), use_bf16)
    if key not in _CACHE:
        _CACHE[key] = _build(n_pos, use_bf16=use_bf16)
    nc = _CACHE[key]

    in_maps = []
    for k in range(NCORE):
        m = dict(consts)
        m.update(per_core[k])
        in_maps.append({kk: np.ascontiguousarray(v, dtype=np.float32)
                        for kk, v in m.items()})
    import os
    trace = bool(int(os.environ.get("KERNEL_TRACE", "0")))
    res = run_bass_kernel_spmd(nc, in_maps, core_ids=list(range(NCORE)),
                               trace=trace)
    global _LAST_RES
    _LAST_RES = res
    outs = [r["out"][:C_CORE] for r in res.results]
    return np.concatenate(outs, 0).astype(np.float32)


_LAST_RES = None
